# revision 5
# baseline (speedup 1.0000x reference)
"""Trainium2 Bass kernel for the BandedJointEncoder problem.

Math: the reference builds, per (batch b, latent z), an upper-bidiagonal
precision Cholesky factor U (diag d_t = softplus(.)+1, superdiag
s_t = softplus(.)) and returns scale_tril = (U^{-1})^T (plus the mean rows).

The inverse of an upper-bidiagonal matrix has the closed form
    V[i,j] = U^{-1}[i,j] = (-1)^{j-i} * (prod_{k=i..j-1} s_k) / (prod_{k=i..j} d_k)
which in log space is a masked rank-1 outer product:
    V[i,j] = (-1)^{j-i} * exp(alpha_j + beta_i),  j >= i
    alpha_j = LS(j) - LD(j+1),  beta_i = LD(i) - LS(i)
with LS/LD the exclusive prefix sums of log s / log d.  So
    scale_tril[p,q] = (-1)^{p+q} * exp(alpha_p + beta_q) for p >= q, else 0.

The per-(b,z) [T,T] triangular solve therefore reduces to prefix scans of
length T plus one masked exp-outer-product expansion, which is purely
memory-bound.  Values decay geometrically away from the diagonal; for f32
anything beyond ~250 subdiagonals underflows to zero, so only a band of
column blocks is computed and written (the runner pre-zeroes output
buffers; see bass2jax.run_bass_via_pjrt / bass_utils.run_bass_kernel_spmd).

Sharding: 64 (b,z) pairs over 8 cores, 8 pairs per core; core k handles
b = k//2 and z in [ (k%2)*8, (k%2)*8+8 ).
"""

import numpy as np

import concourse.bacc as bacc
import concourse.mybir as mybir
from concourse.bass_utils import run_bass_kernel_spmd
from concourse.tile import TileContext

B, T, D = 4, 1024, 64
Z = 16
NCORES = 8
NZ = 8         # z per core
NR = T // 128  # row tiles per matrix
NBLK = 3       # column blocks (128 each) computed per row tile (band width)
BIG = 3.0e38   # clamp for exp overflow in the strictly-upper region
DT = mybir.dt.float32

# packed-constant column layout inside the single "inp" tensor
C_XT = 0                # [0:64,    C_XT   : C_XT+1024]  x[b]^T
C_WP = C_XT + T         # [0:64,    C_WP   : C_WP+24]    permuted W columns
C_BM = C_WP + 3 * NZ    # [0:8,     C_BM]                mean bias
C_BD = C_BM + 1         # [0:8,     C_BD]                d bias
C_BS = C_BD + 1         # [0:8,     C_BS]                s bias
C_ID = C_BS + 1         # [0:8,     C_ID   : C_ID+8]     identity 8x8
C_SEL = C_ID + NZ       # [0:8,     C_SEL  : C_SEL+1024] one-hot selector
C_MSK = C_SEL + NZ * 128  # [0:128, C_MSK  : C_MSK+NBLK*128] sign/tril mask
C_TOT = C_MSK + NBLK * 128

_nc_cache = {}


def _build_nc():
    """Build the SPMD Bass program (identical for all cores)."""
    nc = bacc.Bacc()

    inp = nc.dram_tensor("inp", [128, C_TOT], DT, kind="ExternalInput")
    omean = nc.dram_tensor("omean", [NZ, T], DT, kind="ExternalOutput")
    oscale = nc.dram_tensor("oscale", [NZ * T, T], DT, kind="ExternalOutput")

    AF = mybir.ActivationFunctionType
    OP = mybir.AluOpType

    with TileContext(nc) as tc:
        with (
            tc.tile_pool(name="consts", bufs=1) as consts,
            tc.tile_pool(name="work", bufs=1) as work,
            tc.tile_pool(name="pexp", bufs=3) as pexp,
            tc.tile_pool(name="pout", bufs=4) as pout,
        ):
            ct = consts.tile([128, C_TOT], DT)
            nc.sync.dma_start(ct[:], inp[:])
            xT_t = ct[0:D, C_XT : C_XT + T]
            wp_t = ct[0:D, C_WP : C_WP + 3 * NZ]
            bmean_t = ct[0:NZ, C_BM : C_BM + 1]
            bd_t = ct[0:NZ, C_BD : C_BD + 1]
            bs_t = ct[0:NZ, C_BS : C_BS + 1]
            id8_t = ct[0:NZ, C_ID : C_ID + NZ]
            sel_t = ct[0:NZ, C_SEL : C_SEL + NZ * 128]
            cmask_t = ct[:, C_MSK : C_MSK + NBLK * 128]

            # ---- encoder matmul: [8,3072] = [mean | d-pre | s-pre] ----
            with tc.tile_pool(name="pmm", bufs=1, space="PSUM") as pmm:
                mm = pmm.tile([NZ, 3 * T], DT)
                for g in range(3):  # 0: mean, 1: d, 2: s
                    for piece in range(T // 512):
                        nc.tensor.matmul(
                            mm[:, g * T + piece * 512 : g * T + (piece + 1) * 512],
                            lhsT=wp_t[:, g * NZ : (g + 1) * NZ],
                            rhs=xT_t[:, piece * 512 : (piece + 1) * 512],
                        )

                mean_sb = work.tile([NZ, T], DT)
                nc.scalar.activation(
                    mean_sb[:], mm[:, 0:T], AF.Identity, bias=bmean_t
                )
                nc.sync.dma_start(omean[:], mean_sb[:])

                # softplus(v) = ln(exp(v) + 1); Exp and Ln share one ACT table set
                ed = work.tile([NZ, T], DT)
                nc.scalar.activation(ed[:], mm[:, T : 2 * T], AF.Exp, bias=bd_t)
                es = work.tile([NZ, T], DT)
                nc.scalar.activation(es[:], mm[:, 2 * T : 3 * T], AF.Exp, bias=bs_t)
            spd = work.tile([NZ, T], DT)
            nc.scalar.activation(spd[:], ed[:], AF.Ln, bias=1.0)
            sps = work.tile([NZ, T], DT)
            nc.scalar.activation(sps[:], es[:], AF.Ln, bias=1.0)

            # guard against ln(0) from pathological/junk columns
            nc.vector.tensor_scalar_max(sps[:], sps[:], 1e-35)

            ld = work.tile([NZ, T], DT)
            nc.scalar.activation(ld[:], spd[:], AF.Ln, bias=1.0)  # ln(d) = ln(sp+1)
            ls = work.tile([NZ, T], DT)
            nc.scalar.activation(ls[:], sps[:], AF.Ln, bias=0.0)

            # inclusive prefix sums along t (fp32 internal state)
            cd = work.tile([NZ, T], DT)
            nc.vector.tensor_tensor_scan(cd[:], ld[:], ld[:], 0.0, OP.add, OP.bypass)
            cs = work.tile([NZ, T], DT)
            nc.vector.tensor_tensor_scan(cs[:], ls[:], ls[:], 0.0, OP.add, OP.bypass)

            # alpha = (cs - ls) - cd ; beta = -(alpha + ld)
            t0 = work.tile([NZ, T], DT)
            nc.vector.tensor_sub(t0[:], cs[:], ls[:])
            alpha_t = work.tile([NZ, T], DT)
            nc.vector.tensor_sub(alpha_t[:], t0[:], cd[:])
            t1 = work.tile([NZ, T], DT)
            nc.vector.tensor_add(t1[:], alpha_t[:], ld[:])
            beta_t = work.tile([NZ, T], DT)
            nc.vector.tensor_scalar_mul(beta_t[:], t1[:], -1.0)

            with (
                tc.tile_pool(name="paT", bufs=1, space="PSUM") as ppaT,
                tc.tile_pool(name="pbrep", bufs=2, space="PSUM") as pbrep,
            ):
                # transpose alpha -> per-partition bias columns aT[:, 8r+z]
                paT = ppaT.tile([128, NR * NZ], DT)
                for r in range(NR):
                    nc.tensor.transpose(
                        paT[:, r * NZ : (r + 1) * NZ],
                        alpha_t[:, r * 128 : (r + 1) * 128],
                        id8_t,
                    )
                aT_t = work.tile([128, NR * NZ], DT)
                nc.scalar.copy(aT_t[:], paT[:])

                for zi in range(NZ):
                    # broadcast beta[zi] across 128 partitions via one-hot matmul
                    brep = pbrep.tile([128, T], DT)
                    for piece in range(T // 512):
                        nc.tensor.matmul(
                            brep[:, piece * 512 : (piece + 1) * 512],
                            lhsT=sel_t[:, zi * 128 : (zi + 1) * 128],
                            rhs=beta_t[:, piece * 512 : (piece + 1) * 512],
                        )
                    for r in range(NR):
                        n = min(r, NBLK - 1)
                        c0 = 128 * (r - n)
                        wd = 128 * (n + 1)
                        et = pexp.tile([128, NBLK * 128], DT)
                        nc.scalar.activation(
                            et[:, 0:wd],
                            brep[:, c0 : c0 + wd],
                            AF.Exp,
                            bias=aT_t[:, r * NZ + zi : r * NZ + zi + 1],
                        )
                        ot = pout.tile([128, NBLK * 128], DT)
                        # out = min(exp, BIG) * mask   (kills Inf above diagonal)
                        nc.vector.scalar_tensor_tensor(
                            ot[:, 0:wd],
                            et[:, 0:wd],
                            BIG,
                            cmask_t[:, (NBLK - 1 - n) * 128 : NBLK * 128],
                            OP.min,
                            OP.mult,
                        )
                        row0 = zi * T + r * 128
                        nc.sync.dma_start(
                            oscale[row0 : row0 + 128, c0 : c0 + wd], ot[:, 0:wd]
                        )
    nc.compile()
    return nc


def _host_inputs(x, W, b):
    """Per-core input maps (everything packed into one tensor)."""
    x = np.ascontiguousarray(x, dtype=np.float32)
    W = np.ascontiguousarray(W, dtype=np.float32)
    b = np.ascontiguousarray(b, dtype=np.float32)

    p = np.arange(128)[:, None]
    j = np.arange(NBLK * 128)[None, :]
    cmask = np.where(((p + j) % 2) == 0, np.float32(1.0), np.float32(-1.0))
    t = j[:, (NBLK - 1) * 128 :] - (NBLK - 1) * 128
    last = cmask[:, (NBLK - 1) * 128 :]
    cmask[:, (NBLK - 1) * 128 :] = np.where(t <= p, last, np.float32(0.0))

    sel = np.zeros((NZ, NZ * 128), np.float32)
    for k in range(NZ):
        sel[k, k * 128 : (k + 1) * 128] = 1.0
    id8 = np.eye(NZ, dtype=np.float32)

    in_maps = []
    for k in range(NCORES):
        bidx = k // 2
        z0 = (k % 2) * NZ
        zcols = np.arange(z0, z0 + NZ)
        cols = np.concatenate([zcols, Z + 2 * zcols, Z + 2 * zcols + 1])
        inp = np.zeros((128, C_TOT), np.float32)
        inp[0:D, C_XT : C_XT + T] = x[bidx].T
        inp[0:D, C_WP : C_WP + 3 * NZ] = W[:, cols]
        inp[0:NZ, C_BM] = b[zcols]
        inp[0:NZ, C_BD] = b[Z + 2 * zcols]
        inp[0:NZ, C_BS] = b[Z + 2 * zcols + 1]
        inp[0:NZ, C_ID : C_ID + NZ] = id8
        inp[0:NZ, C_SEL : C_SEL + NZ * 128] = sel
        inp[:, C_MSK : C_MSK + NBLK * 128] = cmask
        in_maps.append({"inp": inp})
    return in_maps


def kernel(x, W, b, _trace=False, _tmpdir=None):
    if "nc" not in _nc_cache:
        _nc_cache["nc"] = _build_nc()
    nc = _nc_cache["nc"]

    in_maps = _host_inputs(x, W, b)
    res = run_bass_kernel_spmd(
        nc,
        in_maps,
        core_ids=list(range(NCORES)),
        trace=_trace,
        tmpdir=_tmpdir,
    )
    _nc_cache["last_results"] = res

    mean_full = np.empty((B, Z, T), np.float32)
    scale_full = np.empty((B, Z, T, T), np.float32)
    for k in range(NCORES):
        bidx = k // 2
        z0 = (k % 2) * NZ
        r = res.results[k]
        mean_full[bidx, z0 : z0 + NZ] = r["omean"]
        scale_full[bidx, z0 : z0 + NZ] = r["oscale"].reshape(NZ, T, T)
    return mean_full, scale_full
